# revision 1
# baseline (speedup 1.0000x reference)
"""Trainium2 Bass kernel for nn_Eq2to2 (Maron et al. equivariant 2->2 layer).

Math (per batch n, with x[n,d,i,j] = inputs[n,i,j,d], W_b = coefs[:,:,b]):
  out[n,i,j,s] = LeakyReLU( sum_d W9[d,s] x[n,d,i,j] + W10[d,s] x[n,d,j,i]
                 + U[n,j,s] + V[n,i,s] + G[n,s] + bias[s]
                 + [i==j] (Dd[n,i,s] + E[n,s] + diag_bias[s]) )
  U = c@W5 + r@W6 + diag@W12, V = c@W7 + r@W8 + diag@W11
  Dd = diag@W0 + r@W2 + c@W3, G = tr@W13 + S@W14, E = tr@W1 + S@W4
  r = row sums, c = col sums, diag = diagonal, tr/S = their totals.

Sharding: 8 cores = (batch n = core//2) x (out-channel half = core%2).

Per core:
  build: transpose x into XT[d, i*128+j] with PE transposes (column sums
    accumulated into PSUM by a second transpose pass, row sums by chunked
    DVE reduces); input arrives host-swizzled [j, i, d] so loads are
    contiguous.
  main: per quad (4 output rows), two dense N=512 matmuls with W
    stationary in [s, j] orientation (float32r fast path); U+V merged
    off-chain on the otherwise-idle GPSIMD; one DVE add folds them into
    the PSUM result; PE transposes back to [j, s]; LeakyReLU fused into
    the PSUM->SBUF move on ACT; one contiguous 128KB DMA per quad.
  diag: the (i,i,:) outputs are recomputed exactly in a tiny [s, i] pass
    and overwrite the main loop's diagonal bytes at the end.
"""

import os
import sys

if "/opt/trn_rl_repo" not in sys.path:
    sys.path.insert(0, "/opt/trn_rl_repo")

import numpy as np

import concourse.bass as bass
import concourse.tile as tile
from concourse import bacc, mybir
from concourse.bass_utils import run_bass_kernel_spmd

B, M, D, S = 4, 128, 128, 128
SH = S // 2          # out channels per core
NB = 15              # basis size
NCORES = 8
F32 = mybir.dt.float32
AF = mybir.ActivationFunctionType
NEG_SLOPE = 0.01

# "f32" (exact) or "f32r" (fp32 storage, reduced-precision PE multiply, 4x
# matmul throughput at N>=256). Applies to the dense per-tile matmuls only.
MM_DTYPE = os.environ.get("EQ2_MM_DTYPE", "f32r")
# dummy matmuls per pair to keep the PE HAM clock-gate warm (0 = off)
WARM_MM = int(os.environ.get("EQ2_WARM", "0"))


def _xtdt():
    if MM_DTYPE == "f32r":
        return mybir.dt.float32r
    if MM_DTYPE == "bf16":
        return mybir.dt.bfloat16
    return F32


def _build_kernel():
    nc = bacc.Bacc(
        "TRN2", target_bir_lowering=False, debug=False, num_devices=NCORES
    )
    # xn is host-swizzled to [j, i, d] so build loads are contiguous
    xn = nc.dram_tensor("xn", [M, M, D], F32, kind="ExternalInput")
    wmat = nc.dram_tensor("wmat", [D, NB * SH], F32, kind="ExternalInput")
    biasv = nc.dram_tensor("biasv", [SH, 1], F32, kind="ExternalInput")
    dbiasv = nc.dram_tensor("dbiasv", [SH, 1], F32, kind="ExternalInput")
    identd = nc.dram_tensor("identd", [M, M], F32, kind="ExternalInput")
    out_t = nc.dram_tensor("out", [M, M, SH], F32, kind="ExternalOutput")

    with tile.TileContext(nc) as tc:
        _kernel_body(tc, nc, xn, wmat, biasv, dbiasv, identd, out_t)

    nc.compile()
    return nc


def _kernel_body(tc, nc, xn, wmat, biasv, dbiasv, identd, out_t):
    with (
        tc.tile_pool(name="const", bufs=1) as constp,
        tc.tile_pool(name="small", bufs=1) as smallp,
        tc.tile_pool(name="xt", bufs=1) as xtp,
    ):
        ident = constp.tile([M, M], F32)
        nc.sync.dma_start(ident[:], identd.ap())
        w_sb = constp.tile([D, NB * SH], F32)
        nc.sync.dma_start(w_sb[:], wmat.ap())
        bias_sb = smallp.tile([SH, 1], F32)
        nc.sync.dma_start(bias_sb[:], biasv.ap())
        dbias_sb = smallp.tile([SH, 1], F32)
        nc.sync.dma_start(dbias_sb[:], dbiasv.ap())

        def w(b):
            return w_sb[:, b * SH:(b + 1) * SH]

        xt = xtp.tile([D, M * M], _xtdt())   # [d, i*128 + j]
        r_sb = smallp.tile([D, M], F32)      # row sums r[n,i,d] as [d, i]
        c_sb = smallp.tile([D, M], F32)      # col sums c[n,j,d] as [d, j]
        diagT = smallp.tile([D, M], F32)     # diag[n,k,d] as [d, k]
        trace_col = smallp.tile([D, 1], F32)
        s_col = smallp.tile([D, 1], F32)
        vb_sb = smallp.tile([SH, M], F32)    # V + G + bias, [s, i]
        dcomb_sb = smallp.tile([SH, M], F32)  # Dd + E + diag_bias, [s, i]
        u_sb = smallp.tile([SH, M], F32)     # U as [s, j]
        w_r = smallp.tile([D, 2 * SH], _xtdt())  # rounded W9|W10 for mains
        wsum_sb = smallp.tile([D, SH], F32)  # W9 + W10 (diag pass)
        idr = smallp.tile([SH, SH], _xtdt())  # identity for out-transposes

        # ---- build phase: transpose x into XT, reduce r/c/diag ----
        NCH, CH = 8, M // 8
        with (
            tc.tile_pool(name="ach", bufs=4) as apool,
            tc.tile_pool(name="pt", bufs=6, space="PSUM") as ptp,
            tc.tile_pool(name="pacc", bufs=1, space="PSUM") as paccp,
        ):
            # warm the PE HAM clock-gate while the first DMAs land
            if True:
                pwu = ptp.tile([M, M], F32, tag="pt")
                for _ in range(14):
                    nc.tensor.matmul(
                        pwu[:], ident[:], ident[:], start=True, stop=True,
                        skip_group_check=True,
                    )
            psum_c = paccp.tile([D, M], F32)
            for k in range(NCH):
                ach = apool.tile([M, CH * D], F32)  # [j, (i_local, d)]
                # xn is [j, i, d]: contiguous 8KB runs per partition
                src = xn.ap()[:, k * CH:(k + 1) * CH, :]
                a3 = ach[:].rearrange("j (i d) -> j i d", i=CH)
                nc.sync.dma_start(a3, src)
                for il in range(CH):
                    i = k * CH + il
                    a_i = a3[:, il, :]  # [j=128, d=128]
                    pt = ptp.tile([D, M], F32)
                    nc.tensor.transpose(pt[:], a_i, ident[:])
                    # col-sum accumulation: psum_c += transpose(a_i)
                    nc.tensor.matmul(
                        psum_c[:], a_i, ident[:],
                        is_transpose=True,
                        start=(i == 0), stop=(i == M - 1),
                    )
                    # PSUM -> SBUF copy, alternating engines
                    dstc = xt[:, i * M:(i + 1) * M]
                    if i % 2 == 0:
                        nc.scalar.activation(dstc, pt[:], AF.Identity)
                    else:
                        nc.vector.tensor_copy(dstc, pt[:])
                # row sums r[n,i,d] as [d,i], fine-grained
                xt3 = xt[:].rearrange("d (i j) -> d i j", i=M)
                h = CH // 2
                for q in range(2):
                    lo = k * CH + q * h
                    nc.vector.reduce_sum(
                        r_sb[:, lo:lo + h],
                        xt3[:, lo:lo + h, :],
                        axis=mybir.AxisListType.X,
                    )

            nc.vector.tensor_copy(c_sb[:], psum_c[:])
            nc.vector.reduce_sum(
                s_col[:], c_sb[:], axis=mybir.AxisListType.X
            )

            # diagonal: strided DMA straight from DRAM ([j,i,d] symmetric)
            diag_nat = smallp.tile([M, D], F32)
            diag_src = bass.AP(xn, 0, [[M * D + D, M], [1, D]])
            nc.sync.dma_start(diag_nat[:], diag_src)
            pdt = ptp.tile([D, M], F32, bufs=1)
            nc.tensor.transpose(pdt[:], diag_nat[:], ident[:])
            nc.scalar.activation(
                diagT[:], pdt[:], AF.Identity, accum_out=trace_col[:]
            )

        # ---- projections of the reduced quantities ----
        with tc.tile_pool(name="proj", bufs=1, space="PSUM") as projp:
            pu = projp.tile([SH, M], F32)
            nc.tensor.matmul(pu[:], w(5), c_sb[:], start=True, stop=False)
            nc.tensor.matmul(pu[:], w(6), r_sb[:], start=False, stop=False)
            nc.tensor.matmul(pu[:], w(12), diagT[:], start=False, stop=True)

            pv = projp.tile([SH, M], F32)
            nc.tensor.matmul(pv[:], w(7), c_sb[:], start=True, stop=False)
            nc.tensor.matmul(pv[:], w(8), r_sb[:], start=False, stop=False)
            nc.tensor.matmul(pv[:], w(11), diagT[:], start=False, stop=True)

            pdd = projp.tile([SH, M], F32)
            nc.tensor.matmul(pdd[:], w(0), diagT[:], start=True, stop=False)
            nc.tensor.matmul(pdd[:], w(2), r_sb[:], start=False, stop=False)
            nc.tensor.matmul(pdd[:], w(3), c_sb[:], start=False, stop=True)

            pge = projp.tile([SH, 2], F32)
            nc.tensor.matmul(
                pge[:, 0:1], w(13), trace_col[:], start=True, stop=False)
            nc.tensor.matmul(
                pge[:, 0:1], w(14), s_col[:], start=False, stop=True)
            nc.tensor.matmul(
                pge[:, 1:2], w(1), trace_col[:], start=True, stop=False)
            nc.tensor.matmul(
                pge[:, 1:2], w(4), s_col[:], start=False, stop=True)

            gb = smallp.tile([SH, 1], F32)
            nc.vector.tensor_add(gb[:], pge[:, 0:1], bias_sb[:])
            ed = smallp.tile([SH, 1], F32)
            nc.vector.tensor_add(ed[:], pge[:, 1:2], dbias_sb[:])
            nc.vector.tensor_scalar_add(vb_sb[:], pv[:], gb[:])
            nc.vector.tensor_scalar_add(dcomb_sb[:], pdd[:], ed[:])
            nc.vector.tensor_copy(u_sb[:], pu[:])
            nc.vector.tensor_copy(w_r[:], w_sb[:, 9 * SH:11 * SH])
            nc.vector.tensor_copy(idr[:], ident[0:SH, 0:SH])
            nc.vector.tensor_add(wsum_sb[:], w(9), w(10))

        # ---- diagonal pass: exact (i,i,:) outputs in [s, i] form ----
        with tc.tile_pool(name="dg", bufs=1, space="PSUM") as dgp:
            pdg = dgp.tile([SH, M], F32)
            nc.tensor.matmul(
                pdg[:], wsum_sb[:], diagT[:], start=True, stop=True
            )
            dtmp = smallp.tile([SH, M], F32)
            nc.vector.tensor_add(dtmp[:], pdg[:], u_sb[:])
            nc.vector.tensor_add(dtmp[:], dtmp[:], vb_sb[:])
            nc.vector.tensor_add(dtmp[:], dtmp[:], dcomb_sb[:])
            pdg2 = dgp.tile([M, SH], F32)
            nc.tensor.transpose(pdg2[:], dtmp[:], ident[0:SH, 0:SH])
            dout = smallp.tile([M, SH], F32)
            nc.scalar.activation(dout[:], pdg2[:], AF.Lrelu, alpha=NEG_SLOPE)

        # ---- main loop: four output rows (one quad) per iteration ----
        xt_mm1 = xt[:].rearrange("d (i j) -> d i j", i=M)
        xt_mm2 = xt[:].rearrange("d (j i) -> d i j", j=M)
        u4 = u_sb[:].unsqueeze(1).broadcast_to([SH, 4, M])
        with (
            tc.tile_pool(name="p1", bufs=4, space="PSUM") as p1pool,
            tc.tile_pool(name="p2", bufs=3, space="PSUM") as p2pool,
            tc.tile_pool(name="tmp", bufs=4) as tmppool,
            tc.tile_pool(name="uq", bufs=3) as uqpool,
            tc.tile_pool(name="osb", bufs=4) as opool,
        ):
            for qd in range(M // 4):
                i0 = 4 * qd
                # off-chain on GPSIMD: uq = U + V columns for this quad
                uq = uqpool.tile([SH, 4 * M], F32)
                vbq = vb_sb[:, i0:i0 + 4].unsqueeze(2).broadcast_to(
                    [SH, 4, M]
                )
                nc.gpsimd.tensor_add(
                    uq[:].rearrange("s (t j) -> s t j", t=4), vbq, u4
                )
                p1 = p1pool.tile([SH, 4 * M], F32)
                p13 = p1[:].rearrange("s (t j) -> s t j", t=4)
                nc.tensor.matmul(
                    p13, w_r[:, 0:SH], xt_mm1[:, i0:i0 + 4, :],
                    start=True, stop=False,
                )
                nc.tensor.matmul(
                    p13, w_r[:, SH:2 * SH], xt_mm2[:, i0:i0 + 4, :],
                    start=False, stop=True,
                )
                # single DVE pass: tmp = psum + (U + V)
                tmp = tmppool.tile([SH, 4 * M], _xtdt())
                nc.vector.tensor_add(tmp[:], p1[:], uq[:])
                p2 = p2pool.tile([M, 4 * SH], _xtdt())
                for t in range(4):
                    nc.tensor.transpose(
                        p2[:, t * SH:(t + 1) * SH],
                        tmp[:, t * M:(t + 1) * M],
                        idr[:],
                    )
                osb = opool.tile([M, 4 * SH], F32)
                # LeakyReLU fused into the PSUM->SBUF move, on ACT
                nc.scalar.activation(
                    osb[:], p2[:], AF.Lrelu, alpha=NEG_SLOPE
                )
                dst = out_t.ap()[i0:i0 + 4, :, :].rearrange("i j s -> j i s")
                nc.sync.dma_start(
                    dst, osb[:].rearrange("j (t s) -> j t s", t=4)
                )

            # overwrite diagonal entries with the exact values
            ddst = bass.AP(out_t, 0, [[M * SH + SH, M], [1, SH]])
            nc.sync.dma_start(ddst, dout[:])


_CACHE = {}


def _get_nc():
    key = MM_DTYPE
    if key not in _CACHE:
        _CACHE[key] = _build_kernel()
    return _CACHE[key]


def make_in_maps(inputs, coefs, bias, diag_bias):
    eye = np.ascontiguousarray(np.eye(M, dtype=np.float32))
    in_maps = []
    for core in range(NCORES):
        n, sh = core // 2, core % 2
        so = sh * SH
        wprep = np.ascontiguousarray(
            coefs[:, so:so + SH, :].transpose(0, 2, 1).reshape(D, NB * SH)
        )
        in_maps.append({
            # host swizzle: [i, j, d] -> [j, i, d] for contiguous DMA
            "xn": np.ascontiguousarray(inputs[n].transpose(1, 0, 2)),
            "wmat": wprep,
            "biasv": np.ascontiguousarray(bias[so:so + SH].reshape(SH, 1)),
            "dbiasv": np.ascontiguousarray(
                diag_bias[so:so + SH].reshape(SH, 1)
            ),
            "identd": eye,
        })
    return in_maps


def kernel(inputs, coefs, bias, diag_bias):
    inputs = np.ascontiguousarray(np.asarray(inputs, dtype=np.float32))
    coefs = np.asarray(coefs, dtype=np.float32)
    bias = np.asarray(bias, dtype=np.float32).reshape(-1)
    diag_bias = np.asarray(diag_bias, dtype=np.float32).reshape(-1)

    nc = _get_nc()
    in_maps = make_in_maps(inputs, coefs, bias, diag_bias)
    # the runtime occasionally reports a transient device-unrecoverable
    # state left over from a previous process; a retry clears it
    last_exc = None
    for attempt in range(3):
        try:
            res = run_bass_kernel_spmd(
                nc, in_maps, core_ids=list(range(NCORES))
            )
            break
        except Exception as e:  # noqa: BLE001
            last_exc = e
            import time as _time
            _time.sleep(10 * (attempt + 1))
    else:
        raise last_exc

    out = np.empty((B, M, M, S), dtype=np.float32)
    for core in range(NCORES):
        n, sh = core // 2, core % 2
        out[n, :, :, sh * SH:(sh + 1) * SH] = res.results[core]["out"]
    return out



# revision 10
# speedup vs baseline: 1.6759x; 1.6759x over previous
"""Trainium2 Bass kernel for nn_Eq2to2 (Maron et al. equivariant 2->2 layer).

Math (per batch n, with x[n,d,i,j] = inputs[n,i,j,d], W_b = coefs[:,:,b]):
  out[n,i,j,s] = LeakyReLU( sum_d W9[d,s] x[n,d,i,j] + W10[d,s] x[n,d,j,i]
                 + U[n,j,s] + V[n,i,s] + G[n,s] + bias[s]
                 + [i==j] (Dd[n,i,s] + E[n,s] + diag_bias[s]) )
  U = c@W5 + r@W6 + diag@W12, V = c@W7 + r@W8 + diag@W11
  Dd = diag@W0 + r@W2 + c@W3, G = tr@W13 + S@W14, E = tr@W1 + S@W4
  r = row sums, c = col sums, diag = diagonal, tr/S = their totals.

Sharding: 8 cores = (batch n = core//2) x (row-half = core%2), with the FULL
out_dim per core. The basis is equivariant under simultaneous row+col
permutation, so the host sends each core a rotated matrix
x'[a,b] = x[(a+off)%128, (b+off)%128] and every core runs the identical
program on rows 0..63 of its rotated view; the host un-rotates the output.

Per core (everything bf16 except PSUM accumulation; tolerance is 2e-2):
  load: x' arrives host-transposed as xt[d, a*128+b] in 8 chunks; as each
    chunk lands the PE accumulates column sums into PSUM via accumulate-copy
    matmuls (identity stationary; also keeps the PE clock-gate warm) and the
    DVE reduces row sums.
  reduced terms: U/V'/Dd projections as [s, 128] via 13 small matmuls.
  main: per quad (4 output rows a), uq = U + V' built on Pool (plus the
    diagonal correction folded into 4 strided columns), two dense N=512
    matmuls (W9|W10 stationary), one DVE add psum+uq -> bf16 tmp, 4 PE
    transposes to [b, (t,s)], LeakyReLU on ACT fused with the PSUM->SBUF
    move, one DMA per quad with contiguous 1KB runs ([b, a_local, s] layout;
    host transposes back).
"""

import os
import sys

if "/opt/trn_rl_repo" not in sys.path:
    sys.path.insert(0, "/opt/trn_rl_repo")

import numpy as np
import ml_dtypes

import concourse.bass as bass
import concourse.tile as tile
from concourse import bacc, mybir
from concourse.bass_utils import run_bass_kernel_spmd

B, M, D, S = 4, 128, 128, 128
RH = M // 2          # rows per core
NB = 15
NCORES = 8
F32 = mybir.dt.float32
BF16 = mybir.dt.float16
AF = mybir.ActivationFunctionType
NEG_SLOPE = 0.01
BF_NP = np.float16

# basis slots used by the projection matmuls, in issue order
PROJ_SLOTS = [5, 6, 12, 7, 8, 11, 0, 2, 3, 13, 14, 1, 4]


def _build_kernel():
    nc = bacc.Bacc(
        "TRN2", target_bir_lowering=False, debug=False, num_devices=NCORES
    )
    # x' as [d, a, b] (rotated per core on host), flattened [d, a*128+b]
    xtd = nc.dram_tensor("xt", [D, M * M], BF16, kind="ExternalInput")
    wmm = nc.dram_tensor("wmm", [D, 2 * S], BF16, kind="ExternalInput")
    wproj = nc.dram_tensor("wproj", [D, 13 * S], BF16, kind="ExternalInput")
    biasv = nc.dram_tensor("biasv", [S, 1], F32, kind="ExternalInput")
    dbiasv = nc.dram_tensor("dbiasv", [S, 1], F32, kind="ExternalInput")
    identd = nc.dram_tensor("identd", [M, M], BF16, kind="ExternalInput")
    # [b, a_local, s]; host transposes to [a, b, s]
    out_t = nc.dram_tensor("out", [M, RH, S], BF16, kind="ExternalOutput")

    with tile.TileContext(nc) as tc:
        _kernel_body(tc, nc, xtd, wmm, wproj, biasv, dbiasv, identd, out_t)

    nc.compile()
    return nc


def _kernel_body(tc, nc, xtd, wmm, wproj, biasv, dbiasv, identd, out_t):
    NCH, CH = 8, M // 8   # chunks, rows per chunk
    with (
        tc.tile_pool(name="const", bufs=1) as constp,
        tc.tile_pool(name="small", bufs=1) as smallp,
        tc.tile_pool(name="xt", bufs=1) as xtp,
    ):
        ident = constp.tile([M, M], BF16)
        nc.sync.dma_start(ident[:], identd.ap())
        wmm_sb = constp.tile([D, 2 * S], BF16)
        nc.sync.dma_start(wmm_sb[:], wmm.ap())
        wproj_sb = constp.tile([D, 13 * S], BF16)
        nc.sync.dma_start(wproj_sb[:], wproj.ap())
        bias_sb = smallp.tile([S, 1], F32)
        nc.sync.dma_start(bias_sb[:], biasv.ap())
        dbias_sb = smallp.tile([S, 1], F32)
        nc.sync.dma_start(dbias_sb[:], dbiasv.ap())

        def wp(k):
            return wproj_sb[:, k * S:(k + 1) * S]

        xt = xtp.tile([D, M * M], BF16)      # [d, a*128+b]
        r32 = smallp.tile([D, M], F32)       # row sums as [d, a]
        rbf = smallp.tile([D, M], BF16)
        cbf = smallp.tile([D, M], BF16)      # col sums as [d, b]
        diagbf = smallp.tile([D, M], BF16)   # diagonal as [d, k]
        trs32 = smallp.tile([D, 2], F32)     # [trace | total]
        trsbf = smallp.tile([D, 2], BF16)
        ctmp = smallp.tile([D, M], F32)
        u_bf = smallp.tile([S, M], BF16)     # U as [s, b]
        vb_bf = smallp.tile([S, M], BF16)    # V + G + bias as [s, a]
        dc_bf = smallp.tile([S, M], BF16)    # Dd + E + diag_bias as [s, a]

        with (
            tc.tile_pool(name="cacc", bufs=1, space="PSUM") as caccp,
            tc.tile_pool(name="warm", bufs=1, space="PSUM") as warmp,
        ):
            # a few dummy transposes to start ramping the PE clock while
            # the first chunk DMA is in flight
            pw = warmp.tile([M, M], BF16)
            for _ in range(6):
                nc.tensor.transpose(
                    pw[:], ident[:], ident[:],
                )
            # column sums accumulate on PE: even rows into cols [0:128],
            # odd rows into cols [128:256] (accumulate-copy, I stationary)
            cps2 = caccp.tile([D, 2 * M], F32)
            for k in range(NCH):
                lo = k * CH * M
                nc.sync.dma_start(
                    xt[:, lo:lo + CH * M], xtd.ap()[:, lo:lo + CH * M]
                )
                for p in range(CH // 2):
                    kk = k * (CH // 2) + p
                    nc.tensor.matmul(
                        cps2[:], ident[:],
                        xt[:, kk * 2 * M:(kk + 1) * 2 * M],
                        start=(kk == 0), stop=(kk == M // 2 - 1),
                    )
                # row sums for this chunk on DVE
                a3 = xt[:, lo:lo + CH * M].rearrange("d (a b) -> d a b", a=CH)
                nc.vector.reduce_sum(
                    r32[:, k * CH:(k + 1) * CH], a3, axis=mybir.AxisListType.X
                )

            # diagonal + totals on Pool, casts on DVE/ACT
            nc.gpsimd.tensor_copy(diagbf[:], xt[:, 0:M * M:M + 1])
            nc.vector.reduce_sum(
                trs32[:, 0:1], diagbf[:], axis=mybir.AxisListType.X
            )
            nc.vector.reduce_sum(
                trs32[:, 1:2], r32[:], axis=mybir.AxisListType.X
            )
            nc.gpsimd.tensor_copy(trsbf[:], trs32[:])
            nc.vector.tensor_copy(rbf[:], r32[:])
            # fold the even/odd column-sum halves
            nc.scalar.activation(ctmp[:], cps2[:, M:2 * M], AF.Identity)
            nc.vector.tensor_add(cbf[:], cps2[:, 0:M], ctmp[:])

        # ---- projections + main loop (PSUM pools coexist) ----
        xt2v = xt[:].rearrange("d (b a) -> d a b", b=M)  # [d,a,b] = x'[d,b,a]
        u4 = u_bf[:].unsqueeze(1).broadcast_to([S, 4, M])
        with (
            tc.tile_pool(name="proj", bufs=1, space="PSUM") as projp,
            tc.tile_pool(name="p1", bufs=3, space="PSUM") as p1pool,
            tc.tile_pool(name="p2", bufs=3, space="PSUM") as p2pool,
            tc.tile_pool(name="tmp", bufs=3) as tmppool,
            tc.tile_pool(name="uq", bufs=3) as uqpool,
            tc.tile_pool(name="osb", bufs=3) as opool,
        ):
            NQ = RH // 4
            p1s = [None] * NQ
            p2s = [None] * NQ
            tmps = [None] * NQ

            def pe_mains(q):
                gi0 = 4 * q
                p1 = p1pool.tile([S, 4 * M], F32)
                nc.tensor.matmul(
                    p1[:], wmm_sb[:, 0:S], xt[:, gi0 * M:(gi0 + 4) * M],
                    start=True, stop=False,
                )
                nc.tensor.matmul(
                    p1[:].rearrange("s (t b) -> s t b", t=4),
                    wmm_sb[:, S:2 * S], xt2v[:, gi0:gi0 + 4, :],
                    start=False, stop=True,
                )
                p1s[q] = p1

            def uv_mains(q):
                gi0 = 4 * q
                uq = uqpool.tile([S, 4 * M], BF16)
                vbq = vb_bf[:, gi0:gi0 + 4].unsqueeze(2).broadcast_to(
                    [S, 4, M]
                )
                nc.gpsimd.tensor_add(
                    uq[:].rearrange("s (t b) -> s t b", t=4), vbq, u4
                )
                # diagonal correction: columns t*128 + (gi0+t)
                dv = uq[:, gi0:gi0 + 3 * (M + 1) + 1:M + 1]
                nc.gpsimd.tensor_add(dv, dv, dc_bf[:, gi0:gi0 + 4])
                tmp = tmppool.tile([S, 4 * M], BF16)
                nc.vector.tensor_add(tmp[:], p1s[q][:], uq[:])
                tmps[q] = tmp

            def backend(q):
                p2 = p2pool.tile([M, 4 * S], BF16)
                tmp = tmps[q]
                for t in range(4):
                    nc.tensor.transpose(
                        p2[:, t * S:(t + 1) * S],
                        tmp[:, t * M:(t + 1) * M],
                        ident[:],
                    )
                osb = opool.tile([M, 4 * S], BF16)
                nc.scalar.activation(
                    osb[:], p2[:], AF.Lrelu, alpha=NEG_SLOPE
                )
                dst = out_t.ap()[:, 4 * q:4 * q + 4, :]
                nc.sync.dma_start(
                    dst, osb[:].rearrange("b (t s) -> b t s", t=4)
                )

            pe_mains(0)
            pe_mains(1)

            # projections of the reduced quantities (one PSUM bank)
            pall = projp.tile([S, 3 * M + 2], F32)
            pu = pall[:, 0:M]
            pv = pall[:, M:2 * M]
            pdd = pall[:, 2 * M:3 * M]
            pge = pall[:, 3 * M:3 * M + 2]
            nc.tensor.matmul(pu, wp(0), cbf[:], start=True, stop=False)
            nc.tensor.matmul(pu, wp(1), rbf[:], start=False, stop=False)
            nc.tensor.matmul(pu, wp(2), diagbf[:], start=False, stop=True)
            nc.tensor.matmul(pv, wp(3), cbf[:], start=True, stop=False)
            nc.tensor.matmul(pv, wp(4), rbf[:], start=False, stop=False)
            nc.tensor.matmul(pv, wp(5), diagbf[:], start=False, stop=True)
            nc.tensor.matmul(pdd, wp(6), diagbf[:], start=True, stop=False)
            nc.tensor.matmul(pdd, wp(7), rbf[:], start=False, stop=False)
            nc.tensor.matmul(pdd, wp(8), cbf[:], start=False, stop=True)
            nc.tensor.matmul(
                pge[:, 0:1], wp(9), trsbf[:, 0:1], start=True, stop=False)
            nc.tensor.matmul(
                pge[:, 0:1], wp(10), trsbf[:, 1:2], start=False, stop=True)
            nc.tensor.matmul(
                pge[:, 1:2], wp(11), trsbf[:, 0:1], start=True, stop=False)
            nc.tensor.matmul(
                pge[:, 1:2], wp(12), trsbf[:, 1:2], start=False, stop=True)

            nc.scalar.activation(u_bf[:], pu, AF.Identity)
            nc.vector.tensor_scalar(
                vb_bf[:], pv, pge[:, 0:1], bias_sb[:, 0:1],
                op0=mybir.AluOpType.add, op1=mybir.AluOpType.add,
            )
            nc.vector.tensor_scalar(
                dc_bf[:], pdd, pge[:, 1:2], dbias_sb[:, 0:1],
                op0=mybir.AluOpType.add, op1=mybir.AluOpType.add,
            )

            # software-pipelined main loop: backend lags mains by 2
            uv_mains(0)
            uv_mains(1)
            for q in range(2, NQ):
                pe_mains(q)
                uv_mains(q)
                backend(q - 2)
            backend(NQ - 2)
            backend(NQ - 1)


_CACHE = {}


def _get_nc():
    if "nc" not in _CACHE:
        _CACHE["nc"] = _build_kernel()
    return _CACHE["nc"]


def make_in_maps(inputs, coefs, bias, diag_bias):
    eye = np.eye(M, dtype=np.float32).astype(BF_NP)
    wmm_np = np.ascontiguousarray(
        np.concatenate([coefs[:, :, 9], coefs[:, :, 10]], axis=1)
    ).astype(BF_NP)
    wproj_np = np.ascontiguousarray(
        np.concatenate([coefs[:, :, b] for b in PROJ_SLOTS], axis=1)
    ).astype(BF_NP)
    bias_np = np.ascontiguousarray(bias.reshape(S, 1))
    dbias_np = np.ascontiguousarray(diag_bias.reshape(S, 1))
    in_maps = []
    for core in range(NCORES):
        n, h = core // 2, core % 2
        off = h * RH
        xd = inputs[n].transpose(2, 0, 1)  # [d, i, j]
        if off:
            xd = np.roll(np.roll(xd, -off, axis=1), -off, axis=2)
        in_maps.append({
            "xt": np.ascontiguousarray(xd.reshape(D, M * M)).astype(BF_NP),
            "wmm": wmm_np,
            "wproj": wproj_np,
            "biasv": bias_np,
            "dbiasv": dbias_np,
            "identd": eye,
        })
    return in_maps


def kernel(inputs, coefs, bias, diag_bias):
    inputs = np.ascontiguousarray(np.asarray(inputs, dtype=np.float32))
    coefs = np.asarray(coefs, dtype=np.float32)
    bias = np.asarray(bias, dtype=np.float32).reshape(-1)
    diag_bias = np.asarray(diag_bias, dtype=np.float32).reshape(-1)

    nc = _get_nc()
    in_maps = make_in_maps(inputs, coefs, bias, diag_bias)
    # the runtime occasionally reports a transient device-unrecoverable
    # state left over from a previous process; a retry clears it
    last_exc = None
    for attempt in range(3):
        try:
            res = run_bass_kernel_spmd(
                nc, in_maps, core_ids=list(range(NCORES))
            )
            break
        except Exception as e:  # noqa: BLE001
            last_exc = e
            import time as _time
            _time.sleep(10 * (attempt + 1))
    else:
        raise last_exc

    out = np.empty((B, M, M, S), dtype=np.float32)
    for core in range(NCORES):
        n, h = core // 2, core % 2
        off = h * RH
        # [b, a, s] -> [a, b, s], then undo the column rotation
        blk = res.results[core]["out"].astype(np.float32).transpose(1, 0, 2)
        if off:
            blk = np.roll(blk, off, axis=1)
        out[n, off:off + RH, :, :] = blk
    return out


# revision 23
# speedup vs baseline: 2.3968x; 1.4301x over previous
"""Trainium2 Bass kernel for nn_Eq2to2 (Maron et al. equivariant 2->2 layer).

Math (per batch n, with x[n,d,i,j] = inputs[n,i,j,d], W_b = coefs[:,:,b]):
  out[n,i,j,s] = LeakyReLU( sum_d W9[d,s] x[n,d,i,j] + W10[d,s] x[n,d,j,i]
                 + U[n,j,s] + V[n,i,s] + G[n,s] + bias[s]
                 + [i==j] (Dd[n,i,s] + E[n,s] + diag_bias[s]) )
  U = c@W5 + r@W6 + diag@W12, V = c@W7 + r@W8 + diag@W11
  Dd = diag@W0 + r@W2 + c@W3, G = tr@W13 + S@W14, E = tr@W1 + S@W4
  r = row sums, c = col sums, diag = diagonal, tr/S = their totals.

Sharding: 8 cores = (batch n = core//2) x (row-half = core%2), with the FULL
out_dim per core. The basis is equivariant under simultaneous row+col
permutation, so the host sends each core a rotated matrix
x'[a,b] = x[(a+off)%128, (b+off)%128] and every core runs the identical
program on rows 0..63 of its rotated view; the host un-rotates the output.

Per core (everything bf16 except PSUM accumulation; tolerance is 2e-2):
  load: x' arrives host-transposed as xt[d, a*128+b] in 8 chunks; as each
    chunk lands the PE accumulates column sums into PSUM via accumulate-copy
    matmuls (identity stationary; also keeps the PE clock-gate warm) and the
    DVE reduces row sums.
  reduced terms: U/V'/Dd projections as [s, 128] via 13 small matmuls.
  main: per quad (4 output rows a), uq = U + V' built on Pool (plus the
    diagonal correction folded into 4 strided columns), two dense N=512
    matmuls (W9|W10 stationary), one DVE add psum+uq -> bf16 tmp, 4 PE
    transposes to [b, (t,s)], LeakyReLU on ACT fused with the PSUM->SBUF
    move, one DMA per quad with contiguous 1KB runs ([b, a_local, s] layout;
    host transposes back).
"""

import os
import sys

if "/opt/trn_rl_repo" not in sys.path:
    sys.path.insert(0, "/opt/trn_rl_repo")

import numpy as np
import ml_dtypes

import concourse.bass as bass
import concourse.tile as tile
from concourse import bacc, mybir
from concourse.bass_utils import run_bass_kernel_spmd

B, M, D, S = 4, 128, 128, 128
RH = M // 2          # rows per core
NB = 15
NCORES = 8
F32 = mybir.dt.float32
BF16 = mybir.dt.float16
F8 = mybir.dt.float8e4
AF = mybir.ActivationFunctionType
NEG_SLOPE = 0.01
BF_NP = np.float16

# basis slots used by the projection matmuls, in issue order
PROJ_SLOTS = [5, 6, 12, 7, 8, 11, 0, 2, 3, 13, 14, 1, 4]
MM2SPLIT = os.environ.get("EQ2_MM2SPLIT", "1") == "1"


def _build_kernel():
    nc = bacc.Bacc(
        "TRN2", target_bir_lowering=False, debug=False, num_devices=NCORES
    )
    # x' as [d, a, b] (rotated per core on host), flattened [d, a*128+b]
    xtd = nc.dram_tensor("xt", [D, M * M], BF16, kind="ExternalInput")
    xt8d = nc.dram_tensor("xt8", [D, M * M], F8, kind="ExternalInput")
    wmm = nc.dram_tensor("wmm", [D, S], BF16, kind="ExternalInput")
    w108 = nc.dram_tensor("w108", [D, S], F8, kind="ExternalInput")
    wproj = nc.dram_tensor("wproj", [D, 13 * S], BF16, kind="ExternalInput")
    biasv = nc.dram_tensor("biasv", [S, 1], F32, kind="ExternalInput")
    dbiasv = nc.dram_tensor("dbiasv", [S, 1], F32, kind="ExternalInput")
    identd = nc.dram_tensor("identd", [M, M], BF16, kind="ExternalInput")
    # [b, a_local, s]; host transposes to [a, b, s]
    out_t = nc.dram_tensor("out", [M, RH, S], BF16, kind="ExternalOutput")

    with tile.TileContext(nc) as tc:
        _kernel_body(tc, nc, xtd, xt8d, wmm, w108, wproj, biasv, dbiasv, identd, out_t)

    nc.compile()
    return nc


def _kernel_body(tc, nc, xtd, xt8d, wmm, w108, wproj, biasv, dbiasv, identd, out_t):
    NCH, CH = 8, M // 8   # chunks, rows per chunk
    with (
        tc.tile_pool(name="const", bufs=1) as constp,
        tc.tile_pool(name="small", bufs=1) as smallp,
        tc.tile_pool(name="xt", bufs=1) as xtp,
    ):
        ident = constp.tile([M, M], BF16)
        wmm_sb = constp.tile([D, S], BF16)
        w108_sb = constp.tile([D, S], F8)
        wproj_sb = constp.tile([D, 13 * S], BF16)
        bias_sb = smallp.tile([S, 1], F32)
        dbias_sb = smallp.tile([S, 1], F32)

        def wp(k):
            return wproj_sb[:, k * S:(k + 1) * S]

        xt = xtp.tile([D, M * M], BF16)      # [d, a*128+b]
        ascr = smallp.tile([D, M], BF16)     # ACT accum scratch
        racc32 = smallp.tile([D, M], F32)
        xt8 = xtp.tile([D, M * M], F8)       # transposed copy [d, b*128+a]
        rbf = smallp.tile([D, M], BF16)      # row sums as [d, a]
        cbf = smallp.tile([D, M], BF16)      # col sums as [d, b]
        diagbf = smallp.tile([D, M], BF16)   # diagonal as [d, k]
        trs32 = smallp.tile([D, 2], F32)     # [trace | total]
        trsbf = smallp.tile([D, 2], BF16)
        ctmp = smallp.tile([D, M], F32)
        u_bf = smallp.tile([S, M], BF16)     # U as [s, b]
        vb_bf = smallp.tile([S, M], BF16)    # V + G + bias as [s, a]
        dc_bf = smallp.tile([S, M], BF16)    # Dd + E + diag_bias as [s, a]

        with (
            tc.tile_pool(name="cacc", bufs=1, space="PSUM") as caccp,
            tc.tile_pool(name="warm", bufs=1, space="PSUM") as warmp,
            tc.tile_pool(name="rtree", bufs=2) as rtree,
        ):
            # a few dummy transposes to start ramping the PE clock while
            # the first chunk DMA is in flight
            pw = warmp.tile([M, M], BF16)
            for _ in range(6):
                nc.tensor.transpose(
                    pw[:], ident[:], ident[:],
                )
            # column sums accumulate on PE: even rows into cols [0:128],
            # odd rows into cols [128:256] (accumulate-copy, I stationary)
            cps2 = caccp.tile([D, 2 * M], F32)
            for k in range(NCH):
                lo = k * CH * M
                eng = nc.sync if k % 2 == 0 else nc.scalar
                if k == 0:
                    H = CH * M // 2
                    eng.dma_start(xt[:, 0:H], xtd.ap()[:, 0:H])
                    eng.dma_start(xt[:, H:2 * H], xtd.ap()[:, H:2 * H])
                else:
                    eng.dma_start(
                        xt[:, lo:lo + CH * M], xtd.ap()[:, lo:lo + CH * M]
                    )
                if k == 0:
                    nc.scalar.dma_start(ident[:], identd.ap())
                for p in range(CH // 2):
                    kk = k * (CH // 2) + p
                    nc.tensor.matmul(
                        cps2[:], ident[:],
                        xt[:, kk * 2 * M:(kk + 1) * 2 * M],
                        start=(kk == 0), stop=(kk == M // 2 - 1),
                    )
                # row sums: most chunks on DVE (sub-chunked for tighter
                # pipelining); chunk 1 on a Pool add-tree to offload DVE
                a3 = xt[:, lo:lo + CH * M].rearrange("d (a b) -> d a b", a=CH)
                def pool_tree(lo_a, n):
                    t3 = rtree.tile([D, n * 64], BF16)
                    v = t3[:].rearrange("d (a b) -> d a b", a=n)
                    src_ = a3[:, lo_a:lo_a + n, :]
                    nc.gpsimd.tensor_add(
                        v, src_[:, :, 0:64], src_[:, :, 64:128]
                    )
                    w = 32
                    while w >= 1:
                        nc.gpsimd.tensor_add(
                            v[:, :, 0:w], v[:, :, 0:w], v[:, :, w:2 * w]
                        )
                        w //= 2
                    with nc.allow_low_precision(reason="fp16 row sums ok"):
                        nc.gpsimd.tensor_copy(
                            rbf[:, k * CH + lo_a:k * CH + lo_a + n],
                            v[:, :, 0:1].squeeze(2),
                        )

                def dve_red(lo_a, n):
                    with nc.allow_low_precision(reason="fp16 row sums ok"):
                        nc.vector.reduce_sum(
                            rbf[:, k * CH + lo_a:k * CH + lo_a + n],
                            a3[:, lo_a:lo_a + n, :],
                            axis=mybir.AxisListType.X,
                        )

                if k in (1, 3):
                    pool_tree(0, CH)
                elif k == 5:
                    pool_tree(0, 8)
                    dve_red(8, 8)
                else:
                    dve_red(0, 8)
                    dve_red(8, 8)

            # second-phase loads: fp8 transposed copy + weights
            H8 = M * M // 4
            for k8 in range(4):
                eng = nc.sync if k8 % 2 == 0 else nc.scalar
                eng.dma_start(
                    xt8[:, k8 * H8:(k8 + 1) * H8],
                    xt8d.ap()[:, k8 * H8:(k8 + 1) * H8],
                )
            nc.sync.dma_start(wmm_sb[:], wmm.ap())
            nc.scalar.dma_start(w108_sb[:], w108.ap())
            nc.sync.dma_start(wproj_sb[:], wproj.ap())
            nc.sync.dma_start(bias_sb[:], biasv.ap())
            nc.sync.dma_start(dbias_sb[:], dbiasv.ap())

            # diagonal + totals on Pool, casts on DVE/ACT
            nc.gpsimd.tensor_copy(diagbf[:], xt[:, 0:M * M:M + 1])
            nc.vector.reduce_sum(
                trs32[:, 0:1], diagbf[:], axis=mybir.AxisListType.X
            )
            nc.vector.reduce_sum(
                trs32[:, 1:2], rbf[:], axis=mybir.AxisListType.X
            )
            nc.gpsimd.tensor_copy(trsbf[:], trs32[:])
            # fold the even/odd column-sum halves
            nc.scalar.activation(ctmp[:], cps2[:, M:2 * M], AF.Identity)
            nc.vector.tensor_add(cbf[:], cps2[:, 0:M], ctmp[:])

        # ---- projections + main loop (PSUM pools coexist) ----
        u4 = u_bf[:].unsqueeze(1).broadcast_to([S, 4, M])
        with (
            tc.tile_pool(name="proj", bufs=1, space="PSUM") as projp,
            tc.tile_pool(name="p1", bufs=3, space="PSUM") as p1pool,
            tc.tile_pool(name="p2", bufs=3, space="PSUM") as p2pool,
            tc.tile_pool(name="tmp", bufs=4) as tmppool,
            tc.tile_pool(name="osb", bufs=3) as opool,
        ):
            NQ = RH // 4
            p1s = [None] * NQ
            p2s = [None] * NQ
            tmps = [None] * NQ

            def pe_mains(q):
                gi0 = 4 * q
                p1 = p1pool.tile([S, 4 * M], F32)
                nc.tensor.matmul(
                    p1[:], wmm_sb[:], xt[:, gi0 * M:(gi0 + 4) * M],
                    start=True, stop=False,
                )
                nc.tensor.matmul(
                    p1[:], w108_sb[:], xt8[:, gi0 * M:(gi0 + 4) * M],
                    start=False, stop=False,
                )
                p1s[q] = p1

            def preu(q):
                # U added into PSUM last (identity stationary, broadcast
                # moving); must follow the u_bf write
                nc.tensor.matmul(
                    p1s[q][:].rearrange("s (t b) -> s t b", t=4),
                    ident[:], u4, start=False, stop=True,
                )

            def uv_mains(q):
                gi0 = 4 * q
                vbq = vb_bf[:, gi0:gi0 + 4].unsqueeze(2).broadcast_to(
                    [S, 4, M]
                )
                tmp = tmppool.tile([S, 4 * M], BF16)
                nc.vector.tensor_add(
                    tmp[:].rearrange("s (t b) -> s t b", t=4),
                    p1s[q][:].rearrange("s (t b) -> s t b", t=4),
                    vbq,
                )
                # diagonal correction: columns t*128 + (gi0+t)
                dv = tmp[:, gi0:gi0 + 3 * (M + 1) + 1:M + 1]
                nc.gpsimd.tensor_add(dv, dv, dc_bf[:, gi0:gi0 + 4])
                tmps[q] = tmp

            def backend(q):
                p2 = p2pool.tile([M, 4 * S], BF16)
                tmp = tmps[q]
                for t in range(4):
                    nc.tensor.transpose(
                        p2[:, t * S:(t + 1) * S],
                        tmp[:, t * M:(t + 1) * M],
                        ident[:],
                    )
                osb = opool.tile([M, 4 * S], BF16)
                nc.scalar.activation(
                    osb[:], p2[:], AF.Lrelu, alpha=NEG_SLOPE
                )
                dst = out_t.ap()[:, 4 * q:4 * q + 4, :]
                nc.sync.dma_start(
                    dst, osb[:].rearrange("b (t s) -> b t s", t=4)
                )

            pe_mains(0)
            pe_mains(1)
            pe_mains(2)

            # projections of the reduced quantities
            puT = projp.tile([S, M], F32)
            pu = puT[:]
            pallB = projp.tile([S, 2 * M + 2], F32)
            pv = pallB[:, 0:M]
            pdd = pallB[:, M:2 * M]
            pge = pallB[:, 2 * M:2 * M + 2]
            nc.tensor.matmul(pu, wp(0), cbf[:], start=True, stop=False)
            nc.tensor.matmul(pu, wp(1), rbf[:], start=False, stop=False)
            nc.tensor.matmul(pu, wp(2), diagbf[:], start=False, stop=True)
            nc.tensor.matmul(pv, wp(3), cbf[:], start=True, stop=False)
            nc.tensor.matmul(pv, wp(4), rbf[:], start=False, stop=False)
            nc.tensor.matmul(pv, wp(5), diagbf[:], start=False, stop=True)
            nc.tensor.matmul(
                pge[:, 0:1], wp(9), trsbf[:, 0:1], start=True, stop=False)
            nc.tensor.matmul(
                pge[:, 0:1], wp(10), trsbf[:, 1:2], start=False, stop=True)
            nc.tensor.matmul(
                pge[:, 1:2], wp(11), trsbf[:, 0:1], start=True, stop=False)
            nc.tensor.matmul(
                pge[:, 1:2], wp(12), trsbf[:, 1:2], start=False, stop=True)
            nc.tensor.matmul(pdd, wp(6), diagbf[:], start=True, stop=False)
            nc.tensor.matmul(pdd, wp(7), rbf[:], start=False, stop=False)
            nc.tensor.matmul(pdd, wp(8), cbf[:], start=False, stop=True)

            nc.scalar.activation(u_bf[:], pu, AF.Identity)
            nc.vector.tensor_scalar(
                vb_bf[:], pv, pge[:, 0:1], bias_sb[:, 0:1],
                op0=mybir.AluOpType.add, op1=mybir.AluOpType.add,
            )
            nc.vector.tensor_scalar(
                dc_bf[:], pdd, pge[:, 1:2], dbias_sb[:, 0:1],
                op0=mybir.AluOpType.add, op1=mybir.AluOpType.add,
            )

            # software-pipelined main loop: backend lags mains by 2
            preu(0)
            preu(1)
            preu(2)
            uv_mains(0)
            uv_mains(1)
            uv_mains(2)
            backend(0)
            for q in range(3, NQ):
                pe_mains(q)
                preu(q)
                uv_mains(q)
                backend(q - 2)
            backend(NQ - 2)
            backend(NQ - 1)


_CACHE = {}


def _get_nc():
    if "nc" not in _CACHE:
        _CACHE["nc"] = _build_kernel()
    return _CACHE["nc"]


def make_in_maps(inputs, coefs, bias, diag_bias):
    import ml_dtypes as _mld
    F8_NP = _mld.float8_e4m3
    eye = np.eye(M, dtype=np.float32).astype(BF_NP)
    wmm_np = np.ascontiguousarray(coefs[:, :, 9]).astype(BF_NP)
    w108_np = np.ascontiguousarray(coefs[:, :, 10]).astype(F8_NP)
    wproj_np = np.ascontiguousarray(
        np.concatenate([coefs[:, :, b] for b in PROJ_SLOTS], axis=1)
    ).astype(BF_NP)
    bias_np = np.ascontiguousarray(bias.reshape(S, 1))
    dbias_np = np.ascontiguousarray(diag_bias.reshape(S, 1))
    in_maps = []
    for core in range(NCORES):
        n, h = core // 2, core % 2
        off = h * RH
        xd = inputs[n].transpose(2, 0, 1)  # [d, i, j]
        if off:
            xd = np.roll(np.roll(xd, -off, axis=1), -off, axis=2)
        in_maps.append({
            "xt": np.ascontiguousarray(xd.reshape(D, M * M)).astype(BF_NP),
            "xt8": np.ascontiguousarray(
                xd.transpose(0, 2, 1).reshape(D, M * M)
            ).astype(F8_NP),
            "wmm": wmm_np,
            "w108": w108_np,
            "wproj": wproj_np,
            "biasv": bias_np,
            "dbiasv": dbias_np,
            "identd": eye,
        })
    return in_maps


def kernel(inputs, coefs, bias, diag_bias):
    inputs = np.ascontiguousarray(np.asarray(inputs, dtype=np.float32))
    coefs = np.asarray(coefs, dtype=np.float32)
    bias = np.asarray(bias, dtype=np.float32).reshape(-1)
    diag_bias = np.asarray(diag_bias, dtype=np.float32).reshape(-1)

    nc = _get_nc()
    in_maps = make_in_maps(inputs, coefs, bias, diag_bias)
    # the runtime occasionally reports a transient device-unrecoverable
    # state left over from a previous process; a retry clears it
    last_exc = None
    for attempt in range(3):
        try:
            res = run_bass_kernel_spmd(
                nc, in_maps, core_ids=list(range(NCORES))
            )
            break
        except Exception as e:  # noqa: BLE001
            last_exc = e
            import time as _time
            _time.sleep(10 * (attempt + 1))
    else:
        raise last_exc

    out = np.empty((B, M, M, S), dtype=np.float32)
    for core in range(NCORES):
        n, h = core // 2, core % 2
        off = h * RH
        # [b, a, s] -> [a, b, s], then undo the column rotation
        blk = res.results[core]["out"].astype(np.float32).transpose(1, 0, 2)
        if off:
            blk = np.roll(blk, off, axis=1)
        out[n, off:off + RH, :, :] = blk
    return out


# revision 24
# speedup vs baseline: 2.4006x; 1.0016x over previous
"""Trainium2 Bass kernel for nn_Eq2to2 (Maron et al. equivariant 2->2 layer).

Math (per batch n, with x[n,d,i,j] = inputs[n,i,j,d], W_b = coefs[:,:,b]):
  out[n,i,j,s] = LeakyReLU( sum_d W9[d,s] x[n,d,i,j] + W10[d,s] x[n,d,j,i]
                 + U[n,j,s] + V[n,i,s] + G[n,s] + bias[s]
                 + [i==j] (Dd[n,i,s] + E[n,s] + diag_bias[s]) )
  U = c@W5 + r@W6 + diag@W12, V = c@W7 + r@W8 + diag@W11
  Dd = diag@W0 + r@W2 + c@W3, G = tr@W13 + S@W14, E = tr@W1 + S@W4
  r = row sums, c = col sums, diag = diagonal, tr/S = their totals.

Sharding: 8 cores = (batch n = core//2) x (row-half = core%2), with the FULL
out_dim per core. The basis is equivariant under simultaneous row+col
permutation, so the host sends each core a rotated matrix
x'[a,b] = x[(a+off)%128, (b+off)%128] and every core runs the identical
program on rows 0..63 of its rotated view; the host un-rotates the output.

Per core (everything bf16 except PSUM accumulation; tolerance is 2e-2):
  load: x' arrives host-transposed as xt[d, a*128+b] in 8 chunks; as each
    chunk lands the PE accumulates column sums into PSUM via accumulate-copy
    matmuls (identity stationary; also keeps the PE clock-gate warm) and the
    DVE reduces row sums.
  reduced terms: U/V'/Dd projections as [s, 128] via 13 small matmuls.
  main: per quad (4 output rows a), uq = U + V' built on Pool (plus the
    diagonal correction folded into 4 strided columns), two dense N=512
    matmuls (W9|W10 stationary), one DVE add psum+uq -> bf16 tmp, 4 PE
    transposes to [b, (t,s)], LeakyReLU on ACT fused with the PSUM->SBUF
    move, one DMA per quad with contiguous 1KB runs ([b, a_local, s] layout;
    host transposes back).
"""

import os
import sys

if "/opt/trn_rl_repo" not in sys.path:
    sys.path.insert(0, "/opt/trn_rl_repo")

import numpy as np
import ml_dtypes

import concourse.bass as bass
import concourse.tile as tile
from concourse import bacc, mybir
from concourse.bass_utils import run_bass_kernel_spmd

B, M, D, S = 4, 128, 128, 128
RH = M // 2          # rows per core
NB = 15
NCORES = 8
F32 = mybir.dt.float32
BF16 = mybir.dt.float16
F8 = mybir.dt.float8e4
AF = mybir.ActivationFunctionType
NEG_SLOPE = 0.01
BF_NP = np.float16

# basis slots used by the projection matmuls, in issue order
PROJ_SLOTS = [5, 6, 12, 7, 8, 11, 0, 2, 3, 13, 14, 1, 4]
MM2SPLIT = os.environ.get("EQ2_MM2SPLIT", "1") == "1"


def _build_kernel():
    nc = bacc.Bacc(
        "TRN2", target_bir_lowering=False, debug=False, num_devices=NCORES
    )
    # x' as [d, a, b] (rotated per core on host), flattened [d, a*128+b]
    xtd = nc.dram_tensor("xt", [D, M * M], BF16, kind="ExternalInput")
    xt8d = nc.dram_tensor("xt8", [D, M * M], F8, kind="ExternalInput")
    wmm = nc.dram_tensor("wmm", [D, S], BF16, kind="ExternalInput")
    w108 = nc.dram_tensor("w108", [D, S], F8, kind="ExternalInput")
    wproj = nc.dram_tensor("wproj", [D, 13 * S], BF16, kind="ExternalInput")
    biasv = nc.dram_tensor("biasv", [S, 1], F32, kind="ExternalInput")
    dbiasv = nc.dram_tensor("dbiasv", [S, 1], F32, kind="ExternalInput")
    identd = nc.dram_tensor("identd", [M, M], BF16, kind="ExternalInput")
    # [b, a_local, s]; host transposes to [a, b, s]
    out_t = nc.dram_tensor("out", [M, RH, S], BF16, kind="ExternalOutput")

    with tile.TileContext(nc) as tc:
        _kernel_body(tc, nc, xtd, xt8d, wmm, w108, wproj, biasv, dbiasv, identd, out_t)

    nc.compile()
    return nc


def _kernel_body(tc, nc, xtd, xt8d, wmm, w108, wproj, biasv, dbiasv, identd, out_t):
    NCH, CH = 8, M // 8   # chunks, rows per chunk
    with (
        tc.tile_pool(name="const", bufs=1) as constp,
        tc.tile_pool(name="small", bufs=1) as smallp,
        tc.tile_pool(name="xt", bufs=1) as xtp,
    ):
        ident = constp.tile([M, M], BF16)
        wmm_sb = constp.tile([D, S], BF16)
        w108_sb = constp.tile([D, S], F8)
        wproj_sb = constp.tile([D, 13 * S], BF16)
        bias_sb = smallp.tile([S, 1], F32)
        dbias_sb = smallp.tile([S, 1], F32)

        def wp(k):
            return wproj_sb[:, k * S:(k + 1) * S]

        xt = xtp.tile([D, M * M], BF16)      # [d, a*128+b]
        ascr = smallp.tile([D, M], BF16)     # ACT accum scratch
        racc32 = smallp.tile([D, M], F32)
        xt8 = xtp.tile([D, M * M], F8)       # transposed copy [d, b*128+a]
        rbf = smallp.tile([D, M], BF16)      # row sums as [d, a]
        cbf = smallp.tile([D, M], BF16)      # col sums as [d, b]
        diagbf = smallp.tile([D, M], BF16)   # diagonal as [d, k]
        trs32 = smallp.tile([D, 2], F32)     # [trace | total]
        trsbf = smallp.tile([D, 2], BF16)
        ctmp = smallp.tile([D, M], F32)
        u_bf = smallp.tile([S, M], BF16)     # U as [s, b]
        vb_bf = smallp.tile([S, M], BF16)    # V + G + bias as [s, a]
        dc_bf = smallp.tile([S, M], BF16)    # Dd + E + diag_bias as [s, a]

        with (
            tc.tile_pool(name="cacc", bufs=1, space="PSUM") as caccp,
            tc.tile_pool(name="warm", bufs=1, space="PSUM") as warmp,
            tc.tile_pool(name="rtree", bufs=2) as rtree,
        ):
            # a few dummy transposes to start ramping the PE clock while
            # the first chunk DMA is in flight
            pw = warmp.tile([M, M], BF16)
            for _ in range(28):
                nc.tensor.transpose(
                    pw[:], ident[:], ident[:],
                )
            # column sums accumulate on PE: even rows into cols [0:128],
            # odd rows into cols [128:256] (accumulate-copy, I stationary)
            cps2 = caccp.tile([D, 2 * M], F32)
            for k in range(NCH):
                lo = k * CH * M
                eng = nc.sync if k % 2 == 0 else nc.scalar
                if k == 0:
                    H = CH * M // 2
                    eng.dma_start(xt[:, 0:H], xtd.ap()[:, 0:H])
                    eng.dma_start(xt[:, H:2 * H], xtd.ap()[:, H:2 * H])
                else:
                    eng.dma_start(
                        xt[:, lo:lo + CH * M], xtd.ap()[:, lo:lo + CH * M]
                    )
                if k == 0:
                    nc.scalar.dma_start(ident[:], identd.ap())
                for p in range(CH // 2):
                    kk = k * (CH // 2) + p
                    nc.tensor.matmul(
                        cps2[:], ident[:],
                        xt[:, kk * 2 * M:(kk + 1) * 2 * M],
                        start=(kk == 0), stop=(kk == M // 2 - 1),
                    )
                if k < 4:
                    for _ in range(2):
                        nc.tensor.transpose(pw[:], ident[:], ident[:])
                # row sums: most chunks on DVE (sub-chunked for tighter
                # pipelining); chunk 1 on a Pool add-tree to offload DVE
                a3 = xt[:, lo:lo + CH * M].rearrange("d (a b) -> d a b", a=CH)
                def pool_tree(lo_a, n):
                    t3 = rtree.tile([D, n * 64], BF16)
                    v = t3[:].rearrange("d (a b) -> d a b", a=n)
                    src_ = a3[:, lo_a:lo_a + n, :]
                    nc.gpsimd.tensor_add(
                        v, src_[:, :, 0:64], src_[:, :, 64:128]
                    )
                    w = 32
                    while w >= 1:
                        nc.gpsimd.tensor_add(
                            v[:, :, 0:w], v[:, :, 0:w], v[:, :, w:2 * w]
                        )
                        w //= 2
                    with nc.allow_low_precision(reason="fp16 row sums ok"):
                        nc.gpsimd.tensor_copy(
                            rbf[:, k * CH + lo_a:k * CH + lo_a + n],
                            v[:, :, 0:1].squeeze(2),
                        )

                def dve_red(lo_a, n):
                    with nc.allow_low_precision(reason="fp16 row sums ok"):
                        nc.vector.reduce_sum(
                            rbf[:, k * CH + lo_a:k * CH + lo_a + n],
                            a3[:, lo_a:lo_a + n, :],
                            axis=mybir.AxisListType.X,
                        )

                if k in (1, 3):
                    pool_tree(0, CH)
                elif k == 5:
                    pool_tree(0, 8)
                    dve_red(8, 8)
                else:
                    dve_red(0, 8)
                    dve_red(8, 8)

            # second-phase loads: fp8 transposed copy + weights
            H8 = M * M // 4
            for k8 in range(4):
                eng = nc.sync if k8 % 2 == 0 else nc.scalar
                eng.dma_start(
                    xt8[:, k8 * H8:(k8 + 1) * H8],
                    xt8d.ap()[:, k8 * H8:(k8 + 1) * H8],
                )
            nc.sync.dma_start(wmm_sb[:], wmm.ap())
            nc.scalar.dma_start(w108_sb[:], w108.ap())
            nc.sync.dma_start(wproj_sb[:], wproj.ap())
            nc.sync.dma_start(bias_sb[:], biasv.ap())
            nc.sync.dma_start(dbias_sb[:], dbiasv.ap())

            # touch Lrelu once so the activation-table load happens now,
            # off the critical path (it costs ~1.3us on first use)
            nc.scalar.activation(ascr[:, 0:8], ascr[:, 8:16], AF.Lrelu,
                                 alpha=NEG_SLOPE)

            # diagonal + totals on Pool, casts on DVE/ACT
            nc.gpsimd.tensor_copy(diagbf[:], xt[:, 0:M * M:M + 1])
            nc.vector.reduce_sum(
                trs32[:, 0:1], diagbf[:], axis=mybir.AxisListType.X
            )
            nc.vector.reduce_sum(
                trs32[:, 1:2], rbf[:], axis=mybir.AxisListType.X
            )
            nc.gpsimd.tensor_copy(trsbf[:], trs32[:])
            # fold the even/odd column-sum halves
            nc.scalar.activation(ctmp[:], cps2[:, M:2 * M], AF.Identity)
            nc.vector.tensor_add(cbf[:], cps2[:, 0:M], ctmp[:])

        # ---- projections + main loop (PSUM pools coexist) ----
        u4 = u_bf[:].unsqueeze(1).broadcast_to([S, 4, M])
        with (
            tc.tile_pool(name="proj", bufs=1, space="PSUM") as projp,
            tc.tile_pool(name="p1", bufs=3, space="PSUM") as p1pool,
            tc.tile_pool(name="p2", bufs=3, space="PSUM") as p2pool,
            tc.tile_pool(name="tmp", bufs=4) as tmppool,
            tc.tile_pool(name="osb", bufs=3) as opool,
        ):
            NQ = RH // 4
            p1s = [None] * NQ
            p2s = [None] * NQ
            tmps = [None] * NQ

            def pe_mains(q):
                gi0 = 4 * q
                p1 = p1pool.tile([S, 4 * M], F32)
                nc.tensor.matmul(
                    p1[:], wmm_sb[:], xt[:, gi0 * M:(gi0 + 4) * M],
                    start=True, stop=False,
                )
                nc.tensor.matmul(
                    p1[:], w108_sb[:], xt8[:, gi0 * M:(gi0 + 4) * M],
                    start=False, stop=False,
                )
                p1s[q] = p1

            def preu(q):
                # U added into PSUM last (identity stationary, broadcast
                # moving); must follow the u_bf write
                nc.tensor.matmul(
                    p1s[q][:].rearrange("s (t b) -> s t b", t=4),
                    ident[:], u4, start=False, stop=True,
                )

            def uv_mains(q):
                gi0 = 4 * q
                vbq = vb_bf[:, gi0:gi0 + 4].unsqueeze(2).broadcast_to(
                    [S, 4, M]
                )
                tmp = tmppool.tile([S, 4 * M], BF16)
                nc.vector.tensor_add(
                    tmp[:].rearrange("s (t b) -> s t b", t=4),
                    p1s[q][:].rearrange("s (t b) -> s t b", t=4),
                    vbq,
                )
                # diagonal correction: columns t*128 + (gi0+t)
                dv = tmp[:, gi0:gi0 + 3 * (M + 1) + 1:M + 1]
                nc.gpsimd.tensor_add(dv, dv, dc_bf[:, gi0:gi0 + 4])
                tmps[q] = tmp

            def backend(q):
                p2 = p2pool.tile([M, 4 * S], BF16)
                tmp = tmps[q]
                for t in range(4):
                    nc.tensor.transpose(
                        p2[:, t * S:(t + 1) * S],
                        tmp[:, t * M:(t + 1) * M],
                        ident[:],
                    )
                osb = opool.tile([M, 4 * S], BF16)
                nc.scalar.activation(
                    osb[:], p2[:], AF.Lrelu, alpha=NEG_SLOPE
                )
                dst = out_t.ap()[:, 4 * q:4 * q + 4, :]
                nc.sync.dma_start(
                    dst, osb[:].rearrange("b (t s) -> b t s", t=4)
                )

            pe_mains(0)
            pe_mains(1)
            pe_mains(2)

            # projections of the reduced quantities
            puT = projp.tile([S, M], F32)
            pu = puT[:]
            pallB = projp.tile([S, 2 * M + 2], F32)
            pv = pallB[:, 0:M]
            pdd = pallB[:, M:2 * M]
            pge = pallB[:, 2 * M:2 * M + 2]
            nc.tensor.matmul(pu, wp(0), cbf[:], start=True, stop=False)
            nc.tensor.matmul(pu, wp(1), rbf[:], start=False, stop=False)
            nc.tensor.matmul(pu, wp(2), diagbf[:], start=False, stop=True)
            nc.tensor.matmul(pv, wp(3), cbf[:], start=True, stop=False)
            nc.tensor.matmul(pv, wp(4), rbf[:], start=False, stop=False)
            nc.tensor.matmul(pv, wp(5), diagbf[:], start=False, stop=True)
            nc.tensor.matmul(
                pge[:, 0:1], wp(9), trsbf[:, 0:1], start=True, stop=False)
            nc.tensor.matmul(
                pge[:, 0:1], wp(10), trsbf[:, 1:2], start=False, stop=True)
            nc.tensor.matmul(
                pge[:, 1:2], wp(11), trsbf[:, 0:1], start=True, stop=False)
            nc.tensor.matmul(
                pge[:, 1:2], wp(12), trsbf[:, 1:2], start=False, stop=True)
            nc.tensor.matmul(pdd, wp(6), diagbf[:], start=True, stop=False)
            nc.tensor.matmul(pdd, wp(7), rbf[:], start=False, stop=False)
            nc.tensor.matmul(pdd, wp(8), cbf[:], start=False, stop=True)

            nc.scalar.activation(u_bf[:], pu, AF.Identity)
            nc.vector.tensor_scalar(
                vb_bf[:], pv, pge[:, 0:1], bias_sb[:, 0:1],
                op0=mybir.AluOpType.add, op1=mybir.AluOpType.add,
            )
            nc.vector.tensor_scalar(
                dc_bf[:], pdd, pge[:, 1:2], dbias_sb[:, 0:1],
                op0=mybir.AluOpType.add, op1=mybir.AluOpType.add,
            )

            # software-pipelined main loop: backend lags mains by 2
            preu(0)
            preu(1)
            preu(2)
            uv_mains(0)
            uv_mains(1)
            uv_mains(2)
            backend(0)
            for q in range(3, NQ):
                pe_mains(q)
                preu(q)
                uv_mains(q)
                backend(q - 2)
            backend(NQ - 2)
            backend(NQ - 1)


_CACHE = {}


def _get_nc():
    if "nc" not in _CACHE:
        _CACHE["nc"] = _build_kernel()
    return _CACHE["nc"]


def make_in_maps(inputs, coefs, bias, diag_bias):
    import ml_dtypes as _mld
    F8_NP = _mld.float8_e4m3
    eye = np.eye(M, dtype=np.float32).astype(BF_NP)
    wmm_np = np.ascontiguousarray(coefs[:, :, 9]).astype(BF_NP)
    w108_np = np.ascontiguousarray(coefs[:, :, 10]).astype(F8_NP)
    wproj_np = np.ascontiguousarray(
        np.concatenate([coefs[:, :, b] for b in PROJ_SLOTS], axis=1)
    ).astype(BF_NP)
    bias_np = np.ascontiguousarray(bias.reshape(S, 1))
    dbias_np = np.ascontiguousarray(diag_bias.reshape(S, 1))
    in_maps = []
    for core in range(NCORES):
        n, h = core // 2, core % 2
        off = h * RH
        xd = inputs[n].transpose(2, 0, 1)  # [d, i, j]
        if off:
            xd = np.roll(np.roll(xd, -off, axis=1), -off, axis=2)
        in_maps.append({
            "xt": np.ascontiguousarray(xd.reshape(D, M * M)).astype(BF_NP),
            "xt8": np.ascontiguousarray(
                xd.transpose(0, 2, 1).reshape(D, M * M)
            ).astype(F8_NP),
            "wmm": wmm_np,
            "w108": w108_np,
            "wproj": wproj_np,
            "biasv": bias_np,
            "dbiasv": dbias_np,
            "identd": eye,
        })
    return in_maps


def kernel(inputs, coefs, bias, diag_bias):
    inputs = np.ascontiguousarray(np.asarray(inputs, dtype=np.float32))
    coefs = np.asarray(coefs, dtype=np.float32)
    bias = np.asarray(bias, dtype=np.float32).reshape(-1)
    diag_bias = np.asarray(diag_bias, dtype=np.float32).reshape(-1)

    nc = _get_nc()
    in_maps = make_in_maps(inputs, coefs, bias, diag_bias)
    # the runtime occasionally reports a transient device-unrecoverable
    # state left over from a previous process; a retry clears it
    last_exc = None
    for attempt in range(3):
        try:
            res = run_bass_kernel_spmd(
                nc, in_maps, core_ids=list(range(NCORES))
            )
            break
        except Exception as e:  # noqa: BLE001
            last_exc = e
            import time as _time
            _time.sleep(10 * (attempt + 1))
    else:
        raise last_exc

    out = np.empty((B, M, M, S), dtype=np.float32)
    for core in range(NCORES):
        n, h = core // 2, core % 2
        off = h * RH
        # [b, a, s] -> [a, b, s], then undo the column rotation
        blk = res.results[core]["out"].astype(np.float32).transpose(1, 0, 2)
        if off:
            blk = np.roll(blk, off, axis=1)
        out[n, off:off + RH, :, :] = blk
    return out
